# revision 11
# baseline (speedup 1.0000x reference)
"""GCN (2x GCNConv + GraphNorm + ReLU, MLP head) on 8 TRN2 NeuronCores.

Sharding: destination-node ranges across the 8 cores. Per layer each core
computes its shard of h = (dinv * x) @ W (bf16), AllGathers the full node
table into DRAM, DMA-gathers the source rows of its own (dest-sorted) edges
and segment-sums them with one-hot matmuls on the TensorEngine, accumulating
128-dest windows in PSUM. GraphNorm statistics go through small AllReduces.
All data-dependent structure (gather indices, one-hot columns) is carried by
input tensors so a single program serves all 8 cores.
"""

import math
from dataclasses import dataclass, field

import ml_dtypes
import numpy as np

import concourse.bacc as bacc
import concourse.bass as bass
import concourse.mybir as mybir
import concourse.tile as tile
from concourse.bass_utils import run_bass_kernel_spmd

F32 = mybir.dt.float32
BF16 = mybir.dt.bfloat16
I16 = mybir.dt.int16

AF = mybir.ActivationFunctionType
ALU = mybir.AluOpType

NCORES = 8
NQUAD = 4
D = 128
EPS = 1e-5


@dataclass
class Cfg:
    N: int = 100000
    CH: int = 8  # gather chunk, in 128-edge tiles (num_idxs<=1024 for single_packet)
    NLOC: int = field(init=False)
    NLOC_PAD: int = field(init=False)
    W: int = field(init=False)  # dest windows per core
    QROWS: int = field(init=False)
    TROWS: int = field(init=False)

    def __post_init__(self):
        assert self.N % NCORES == 0
        self.NLOC = self.N // NCORES
        self.W = (self.NLOC + 127) // 128
        self.NLOC_PAD = self.W * 128
        self.QROWS = (NCORES // NQUAD) * self.NLOC_PAD
        self.TROWS = NCORES * self.NLOC_PAD
        assert self.QROWS <= 32768


def preprocess(cfg: Cfg, edge_index: np.ndarray):
    """Sort/bucket edges; build per-core idx + doff streams and a shared
    (core-invariant) window/bucket tile schedule."""
    N, NLOC, NLOC_PAD, W = cfg.N, cfg.NLOC, cfg.NLOC_PAD, cfg.W
    row = edge_index[0].astype(np.int64)
    col = edge_index[1].astype(np.int64)
    loops = np.arange(N, dtype=np.int64)
    row = np.concatenate([row, loops])
    col = np.concatenate([col, loops])

    deg = np.bincount(col, minlength=N).astype(np.float64)
    dinv = (1.0 / np.sqrt(np.maximum(deg, 1.0))).astype(np.float32)

    src_core = row // NLOC
    trow = src_core * NLOC_PAD + (row - src_core * NLOC)  # table row of source
    quad = trow // cfg.QROWS
    qidx = (trow - quad * cfg.QROWS).astype(np.int64)

    dest_core = col // NLOC
    ld = col - dest_core * NLOC  # local dest id
    win = ld // 128
    doff = (ld - win * 128).astype(np.int64)

    # per (core, bucket, window) edge lists (dest-sorted within not required
    # beyond window grouping since the one-hot handles arbitrary order)
    per_core = []
    cnt = np.zeros((NCORES, NQUAD, W), dtype=np.int64)
    for c in range(NCORES):
        m = dest_core == c
        order = np.argsort(quad[m] * W + win[m], kind="stable")
        q_s, w_s = quad[m][order], win[m][order]
        per_core.append(
            dict(qidx=qidx[m][order], doff=doff[m][order], q=q_s, w=w_s)
        )
        np.add.at(cnt[c], (q_s, w_s), 1)

    # shared schedule: tiles per (bucket, window) = max over cores
    K = np.ceil(cnt / 128.0).astype(np.int64).max(axis=0)  # [NQUAD, W]
    T_b = K.sum(axis=1)  # tiles per bucket
    CH = cfg.CH
    T_b_pad = ((T_b + CH - 1) // CH) * CH
    T2 = int(K.sum())  # total matmul slots

    # slot schedule (shared): window-major, bucket-minor
    sched = []  # (w, b, k_in_bucketwindow)
    for w in range(W):
        for b in range(NQUAD):
            for k in range(int(K[b, w])):
                sched.append((w, b))

    ins = []
    for c in range(NCORES):
        pc = per_core[c]
        # offsets of each (b, w) group in this core's sorted stream
        starts = np.zeros((NQUAD, W + 1), dtype=np.int64)
        for b in range(NQUAD):
            for w in range(W):
                starts[b, w + 1] = starts[b, w] + cnt[c, b, w]
        base_b = np.concatenate([[0], np.cumsum(starts[:, -1])])

        idx_streams = []
        doff_slots = np.full((int(T2), 128), -1.0, dtype=np.float32)
        slot_of = {}
        s = 0
        for w in range(W):
            for b in range(NQUAD):
                for k in range(int(K[b, w])):
                    slot_of[(b, w, k)] = s
                    s += 1
        for b in range(NQUAD):
            stream = np.zeros(int(T_b_pad[b]) * 128, dtype=np.int16)
            pos = 0
            for w in range(W):
                lo = base_b[b] + starts[b, w]
                hi = base_b[b] + starts[b, w + 1]
                n = hi - lo
                tiles = int(K[b, w])
                qi = pc["qidx"][lo:hi]
                dof = pc["doff"][lo:hi]
                buf = np.zeros(tiles * 128, dtype=np.int16)
                buf[:n] = qi.astype(np.int16)
                stream[pos : pos + tiles * 128] = buf
                dbuf = np.full(tiles * 128, -1.0, dtype=np.float32)
                dbuf[:n] = dof.astype(np.float32)
                for k in range(tiles):
                    doff_slots[slot_of[(b, w, k)]] = dbuf[k * 128 : (k + 1) * 128]
                pos += tiles * 128
            idx_streams.append(stream)

        # wrapped idx layout [128, len/16]: IDX[p, j] = stream[j*16 + p%16]
        core_in = {}
        for b in range(NQUAD):
            st = idx_streams[b]
            wrapped = st.reshape(-1, 16).T  # [16, len/16]
            core_in[f"idx{b}"] = np.tile(wrapped, (8, 1)).copy()
        core_in["doff"] = doff_slots.T.copy()  # [128, T2] f32
        dl = np.zeros(NLOC_PAD, dtype=np.float32)
        dl[:NLOC] = dinv[c * NLOC : (c + 1) * NLOC]
        core_in["dinvw"] = dl.reshape(W, 128).T.copy()  # [128, W]
        mk = np.zeros(NLOC_PAD, dtype=np.float32)
        mk[:NLOC] = 1.0
        core_in["maskw"] = mk.reshape(W, 128).T.copy()
        ins.append(core_in)

    meta = dict(K=K, T_b=T_b, T_b_pad=T_b_pad, T2=T2, sched=sched, dinv=dinv)
    return ins, meta


def build(cfg: Cfg, meta, lin1b: float) -> bacc.Bacc:
    N, NLOC_PAD, W, CH = cfg.N, cfg.NLOC_PAD, cfg.W, cfg.CH
    K, T_b, T_b_pad, T2 = meta["K"], meta["T_b"], meta["T_b_pad"], meta["T2"]

    nc = bacc.Bacc("TRN2", target_bir_lowering=False, debug=False, num_devices=NCORES, num_swdge_queues=4)

    # --- I/O ---
    X = nc.dram_tensor("x", [NLOC_PAD, D], F32, kind="ExternalInput")
    IDX = [
        nc.dram_tensor(f"idx{b}", [128, int(T_b_pad[b]) * 8], I16, kind="ExternalInput")
        for b in range(NQUAD)
    ]
    DOFF = nc.dram_tensor("doff", [128, T2], F32, kind="ExternalInput")
    DINVW = nc.dram_tensor("dinvw", [128, W], F32, kind="ExternalInput")
    MASKW = nc.dram_tensor("maskw", [128, W], F32, kind="ExternalInput")
    IOTA = nc.dram_tensor("iota", [128, 128], BF16, kind="ExternalInput")
    IDENT = nc.dram_tensor("ident", [128, 128], F32, kind="ExternalInput")
    ONESROW = nc.dram_tensor("onesrow", [1, 128], F32, kind="ExternalInput")
    ONESCOL = nc.dram_tensor("onescol", [128, 1], F32, kind="ExternalInput")
    WMAT = [
        nc.dram_tensor(f"w{l}", [D, D], F32, kind="ExternalInput") for l in range(2)
    ]
    # graphnorm params as columns [128,1]; lin weights
    GN_A = [nc.dram_tensor(f"gn{l}_a", [D, 1], F32, kind="ExternalInput") for l in range(2)]
    GN_W = [nc.dram_tensor(f"gn{l}_w", [D, 1], F32, kind="ExternalInput") for l in range(2)]
    GN_B = [nc.dram_tensor(f"gn{l}_b", [D, 1], F32, kind="ExternalInput") for l in range(2)]
    BCONV = [nc.dram_tensor(f"b{l}", [D, 1], F32, kind="ExternalInput") for l in range(2)]
    LIN0 = nc.dram_tensor("lin0_w", [D, D], F32, kind="ExternalInput")
    LIN0B = nc.dram_tensor("lin0_b", [D, 1], F32, kind="ExternalInput")
    LIN1 = nc.dram_tensor("lin1_w", [D, 1], F32, kind="ExternalInput")
    OUT = nc.dram_tensor("out", [NLOC_PAD, 1], F32, kind="ExternalOutput")

    # --- internal DRAM ---
    SHARD = nc.dram_tensor("shard", [NLOC_PAD, D], BF16)
    TABLE = nc.dram_tensor("table", [cfg.TROWS, D], BF16, addr_space="Shared")
    RS_IN = nc.dram_tensor("rs_in", [D, 1], F32)
    RS_OUT = nc.dram_tensor("rs_out", [D, 1], F32, addr_space="Shared")
    RS_IN2 = nc.dram_tensor("rs_in2", [D, 1], F32)
    RS_OUT2 = nc.dram_tensor("rs_out2", [D, 1], F32, addr_space="Shared")

    rg = [list(range(NCORES))]

    with tile.TileContext(nc) as tc:
        import contextlib

        ctx = contextlib.ExitStack()
        with ctx:
            sb = ctx.enter_context(tc.tile_pool(name="sb", bufs=1))
            x_sb = sb.tile([128, W * D], F32, tag="x")  # [p, w*128+d]
            stage = sb.tile([128, W * D], BF16, tag="stage")
            doff_sb = sb.tile([128, T2], F32, tag="doff")
            dinv_sb = sb.tile([128, W], F32, tag="dinv")
            mask_sb = sb.tile([128, W], F32, tag="mask")
            iota_sb = sb.tile([128, 128], BF16, tag="iota")
            ident_sb = sb.tile([128, 128], F32, tag="ident")
            onesrow_sb = sb.tile([1, 128], F32, tag="onesrow")
            onescol_sb = sb.tile([128, 1], F32, tag="onescol")
            w_sb = [sb.tile([D, D], F32, tag=f"w{l}", name=f"w{l}_sb") for l in range(2)]
            gna_sb = [sb.tile([D, 1], F32, tag=f"gna{l}", name=f"gna{l}_sb") for l in range(2)]
            gnw_sb = [sb.tile([D, 1], F32, tag=f"gnw{l}", name=f"gnw{l}_sb") for l in range(2)]
            gnb_sb = [sb.tile([D, 1], F32, tag=f"gnb{l}", name=f"gnb{l}_sb") for l in range(2)]
            bconv_sb = [sb.tile([D, 1], F32, tag=f"bc{l}", name=f"bc{l}_sb") for l in range(2)]
            lin0_sb = sb.tile([D, D], F32, tag="lin0")
            lin0b_sb = sb.tile([D, 1], F32, tag="lin0b")
            lin1_sb = sb.tile([D, 1], F32, tag="lin1")
            idx_sb = [
                sb.tile([128, int(T_b_pad[b]) * 8], I16, tag=f"idx{b}", name=f"idx{b}_sb")
                for b in range(NQUAD)
            ]
            out_sb = sb.tile([128, W], F32, tag="outsb")

            nc.sync.dma_start(x_sb[:].rearrange("p (w d) -> p w d", w=W), X.ap().rearrange("(w p) d -> p w d", p=128))
            nc.sync.dma_start(doff_sb[:], DOFF[:])
            nc.sync.dma_start(dinv_sb[:], DINVW[:])
            nc.sync.dma_start(mask_sb[:], MASKW[:])
            nc.sync.dma_start(iota_sb[:], IOTA[:])
            nc.sync.dma_start(ident_sb[:], IDENT[:])
            nc.sync.dma_start(onesrow_sb[:], ONESROW[:])
            nc.sync.dma_start(onescol_sb[:], ONESCOL[:])
            for l in range(2):
                nc.sync.dma_start(w_sb[l][:], WMAT[l][:])
                nc.sync.dma_start(gna_sb[l][:], GN_A[l][:])
                nc.sync.dma_start(gnw_sb[l][:], GN_W[l][:])
                nc.sync.dma_start(gnb_sb[l][:], GN_B[l][:])
                nc.sync.dma_start(bconv_sb[l][:], BCONV[l][:])
            nc.sync.dma_start(lin0_sb[:], LIN0[:])
            nc.sync.dma_start(lin0b_sb[:], LIN0B[:])
            nc.sync.dma_start(lin1_sb[:], LIN1[:])
            for b in range(NQUAD):
                nc.sync.dma_start(idx_sb[b][:], IDX[b][:])

            ps_t = ctx.enter_context(tc.tile_pool(name="ps_t", bufs=2, space="PSUM"))
            ps_h = ctx.enter_context(tc.tile_pool(name="ps_h", bufs=2, space="PSUM"))
            ps_w = ctx.enter_context(tc.tile_pool(name="ps_w", bufs=3, space="PSUM"))
            ps_s = ctx.enter_context(tc.tile_pool(name="ps_s", bufs=1, space="PSUM"))
            sp = ctx.enter_context(tc.tile_pool(name="sp", bufs=4))
            gst = [
                ctx.enter_context(tc.tile_pool(name=f"g{b}", bufs=2))
                for b in range(NQUAD)
            ]

            def bcast_row(src_col, scratch_tag):
                """[128,1] f32 column -> [128,128] tile with out[p,d]=src[d]."""
                tp = ps_t.tile([1, 128], F32, tag="tp", name="tp_row")
                nc.tensor.transpose(tp[:], src_col, ident_sb[:])
                row = sp.tile([1, 128], F32, tag=scratch_tag + "_row", name=scratch_tag + "_row")
                nc.scalar.activation(row[:], tp[:], AF.Copy)
                bc = ps_t.tile([128, 128], F32, tag="tp", name="tp_bc")
                nc.tensor.matmul(bc[:], onesrow_sb[:], row[:], start=True, stop=True)
                t = sp.tile([128, 128], F32, tag=scratch_tag, name=scratch_tag)
                nc.scalar.activation(t[:], bc[:], AF.Copy)
                return t

            def prologue(layer):
                """x_sb -> scale by dinv -> @W -> bf16 stage -> SHARD -> AllGather."""
                for w in range(W):
                    xw = x_sb[:, w * D : (w + 1) * D]
                    xs = sp.tile([128, D], F32, tag="p_xs", name="p_xs")
                    nc.vector.tensor_scalar_mul(xs[:], xw, dinv_sb[:, w : w + 1])
                    tp = ps_t.tile([128, D], F32, tag="tp", name="p_tp")
                    nc.tensor.transpose(tp[:], xs[:], ident_sb[:])
                    xT = sp.tile([128, D], F32, tag="p_xT", name="p_xT")
                    nc.scalar.activation(xT[:], tp[:], AF.Copy)
                    hp = ps_h.tile([128, D], F32, tag="hp", name="p_hp")
                    nc.tensor.matmul(hp[:], xT[:], w_sb[layer][:], start=True, stop=True)
                    nc.scalar.activation(
                        stage[:, w * D : (w + 1) * D], hp[:], AF.Copy
                    )
                nc.sync.dma_start(
                    SHARD.ap().rearrange("(w p) d -> p w d", p=128),
                    stage[:].rearrange("p (w d) -> p w d", w=W),
                )
                nc.gpsimd.collective_compute(
                    "AllGather",
                    ALU.bypass,
                    replica_groups=rg,
                    ins=[SHARD.ap().opt()],
                    outs=[TABLE.ap().opt()],
                )

            def allreduce(col_ap, bounce_in, bounce_out, tag):
                t = sp.tile([D, 1], F32, tag=tag, name=tag + "_t")
                nc.scalar.activation(t[:], col_ap, AF.Copy)
                nc.sync.dma_start(bounce_in[:], t[:])
                nc.gpsimd.collective_compute(
                    "AllReduce",
                    ALU.add,
                    replica_groups=rg,
                    ins=[bounce_in.ap().opt()],
                    outs=[bounce_out.ap().opt()],
                )
                r = sp.tile([D, 1], F32, tag=tag + "_r", name=tag + "_r")
                nc.sync.dma_start(r[:], bounce_out[:])
                return r

            def gather_and_aggregate(layer):
                """window-major one-hot matmul accumulation; drains into x_sb."""
                tile_ctr = [0] * NQUAD
                chunk_tiles = [dict() for _ in range(NQUAD)]
                slot = 0
                for w in range(W):
                    nslots = int(K[:, w].sum())
                    pw = ps_w.tile([128, D], F32, tag="agg", name="agg_pw")
                    si = 0
                    for b in range(NQUAD):
                        for k in range(int(K[b, w])):
                            t = tile_ctr[b]
                            cidx = t // CH
                            if cidx not in chunk_tiles[b]:
                                g = gst[b].tile([128, CH, D], BF16, tag="g", name=f"g{b}_t")
                                nidx = CH * 128
                                nc.gpsimd.dma_gather(
                                    g[:],
                                    TABLE.ap()[
                                        b * cfg.QROWS : (b + 1) * cfg.QROWS, :
                                    ],
                                    idx_sb[b][:, cidx * CH * 8 : (cidx + 1) * CH * 8],
                                    nidx,
                                    nidx,
                                    D,
                                    queue_num=b,
                                )
                                chunk_tiles[b] = {cidx: g}  # keep only latest
                            g = chunk_tiles[b][cidx]
                            s_t = sp.tile([128, 128], BF16, tag="s_onehot", name="s_onehot")
                            nc.vector.tensor_scalar(
                                s_t[:],
                                iota_sb[:],
                                doff_sb[:, slot : slot + 1],
                                None,
                                op0=ALU.is_equal,
                            )
                            nc.tensor.matmul(
                                pw[:],
                                s_t[:],
                                g[:, t % CH, :],
                                start=(si == 0),
                                stop=(si == nslots - 1),
                            )
                            tile_ctr[b] += 1
                            slot += 1
                            si += 1
                    # drain: agg = psum * dinv  (pad dests get dinv 0)
                    nc.vector.tensor_scalar_mul(
                        x_sb[:, w * D : (w + 1) * D], pw[:], dinv_sb[:, w : w + 1]
                    )

            def graphnorm_relu(layer):
                # S1: column sums -> mean
                sps = ps_s.tile([128, 1], F32, tag="stats", name="stats_ps")
                for w in range(W):
                    nc.tensor.matmul(
                        sps[:],
                        x_sb[:, w * D : (w + 1) * D],
                        onescol_sb[:],
                        start=(w == 0),
                        stop=(w == W - 1),
                    )
                gsum = allreduce(sps[:], RS_IN, RS_OUT, "ar_mean")
                # m2 = a*(mean + b0) - b0  (columns)
                m2 = sp.tile([D, 1], F32, tag="m2", name="m2")
                nc.vector.tensor_scalar(
                    m2[:], gsum[:], 1.0 / N, None, op0=ALU.mult
                )
                nc.vector.tensor_add(m2[:], m2[:], bconv_sb[layer][:])
                nc.vector.tensor_mul(m2[:], m2[:], gna_sb[layer][:])
                nc.vector.tensor_sub(m2[:], m2[:], bconv_sb[layer][:])
                m2t = bcast_row(m2[:], "m2bc")
                # S2: c = (agg - m2) * mask ; var = sum(c^2)
                vps = ps_s.tile([128, 1], F32, tag="stats", name="stats2_ps")
                for w in range(W):
                    xw = x_sb[:, w * D : (w + 1) * D]
                    nc.vector.tensor_sub(xw, xw, m2t[:])
                    nc.vector.tensor_scalar_mul(xw, xw, mask_sb[:, w : w + 1])
                    sq = sp.tile([128, D], F32, tag="sq", name="sq")
                    nc.vector.tensor_mul(sq[:], xw, xw)
                    nc.tensor.matmul(
                        vps[:],
                        sq[:],
                        onescol_sb[:],
                        start=(w == 0),
                        stop=(w == W - 1),
                    )
                gvar = allreduce(vps[:], RS_IN2, RS_OUT2, "ar_var")
                vs = sp.tile([D, 1], F32, tag="vs", name="vs")
                nc.vector.tensor_scalar(
                    vs[:], gvar[:], 1.0 / N, EPS, op0=ALU.mult, op1=ALU.add
                )
                rc = sp.tile([D, 1], F32, tag="rc", name="rc")
                nc.vector.reciprocal(rc[:], vs[:])
                rstd = sp.tile([D, 1], F32, tag="rstd", name="rstd")
                nc.scalar.activation(rstd[:], rc[:], AF.Sqrt)
                f = sp.tile([D, 1], F32, tag="fcol", name="fcol")
                nc.vector.tensor_mul(f[:], rstd[:], gnw_sb[layer][:])
                ft = bcast_row(f[:], "fbc")
                gt = bcast_row(gnb_sb[layer][:], "gbc")
                # S3: x = relu(c*f + gb)
                for w in range(W):
                    xw = x_sb[:, w * D : (w + 1) * D]
                    nc.vector.tensor_mul(xw, xw, ft[:])
                    nc.vector.tensor_add(xw, xw, gt[:])
                    nc.scalar.activation(xw, xw, AF.Relu)

            def mlp_head():
                b0t = bcast_row(lin0b_sb[:], "l0bc")
                for w in range(W):
                    xw = x_sb[:, w * D : (w + 1) * D]
                    tp = ps_t.tile([128, D], F32, tag="tp", name="m_tp")
                    nc.tensor.transpose(tp[:], xw, ident_sb[:])
                    xT = sp.tile([128, D], F32, tag="m_xT", name="m_xT")
                    nc.scalar.activation(xT[:], tp[:], AF.Copy)
                    yp = ps_h.tile([128, D], F32, tag="hp", name="m_yp")
                    nc.tensor.matmul(yp[:], xT[:], lin0_sb[:], start=True, stop=True)
                    y = sp.tile([128, D], F32, tag="m_y", name="m_y")
                    nc.vector.tensor_add(y[:], yp[:], b0t[:])
                    nc.scalar.activation(y[:], y[:], AF.Relu)
                    tp2 = ps_t.tile([128, D], F32, tag="tp", name="m_tp2")
                    nc.tensor.transpose(tp2[:], y[:], ident_sb[:])
                    yT = sp.tile([128, D], F32, tag="m_yT", name="m_yT")
                    nc.scalar.activation(yT[:], tp2[:], AF.Copy)
                    op = ps_h.tile([128, 1], F32, tag="hp", name="m_op")
                    nc.tensor.matmul(op[:], yT[:], lin1_sb[:], start=True, stop=True)
                    nc.vector.tensor_scalar_add(
                        out_sb[:, w : w + 1], op[:], lin1b
                    )
                nc.sync.dma_start(
                    OUT.ap().rearrange("(w p) one -> p w one", p=128),
                    out_sb[:].rearrange("p (w one) -> p w one", one=1),
                )

            for layer in range(2):
                prologue(layer)
                gather_and_aggregate(layer)
                graphnorm_relu(layer)
            mlp_head()

    nc.compile()
    return nc


def _make_const_inputs(cfg: Cfg, weights: dict):
    """Inputs identical on all cores."""
    c = {}
    c["iota"] = np.broadcast_to(
        np.arange(128, dtype=np.float32), (128, 128)
    ).astype(ml_dtypes.bfloat16)
    c["ident"] = np.eye(128, dtype=np.float32)
    c["onesrow"] = np.ones((1, 128), np.float32)
    c["onescol"] = np.ones((128, 1), np.float32)
    c["w0"] = np.asarray(weights["W0"], np.float32)
    c["w1"] = np.asarray(weights["W1"], np.float32)
    for l in range(2):
        c[f"gn{l}_a"] = np.asarray(weights[f"gn{l}_a"], np.float32).reshape(D, 1)
        c[f"gn{l}_w"] = np.asarray(weights[f"gn{l}_w"], np.float32).reshape(D, 1)
        c[f"gn{l}_b"] = np.asarray(weights[f"gn{l}_b"], np.float32).reshape(D, 1)
        c[f"b{l}"] = np.asarray(weights[f"b{l}"], np.float32).reshape(D, 1)
    c["lin0_w"] = np.asarray(weights["lin0_w"], np.float32)
    c["lin0_b"] = np.asarray(weights["lin0_b"], np.float32).reshape(D, 1)
    c["lin1_w"] = np.asarray(weights["lin1_w"], np.float32).reshape(D, 1)
    return c


def run(cfg: Cfg, x, edge_index, weights, trace=False):
    ins, meta = preprocess(cfg, edge_index)
    consts = _make_const_inputs(cfg, weights)
    x = np.asarray(x, np.float32)
    in_maps = []
    for c in range(NCORES):
        m = dict(ins[c])
        m.update(consts)
        xs = np.zeros((cfg.NLOC_PAD, D), np.float32)
        xs[: cfg.NLOC] = x[c * cfg.NLOC : (c + 1) * cfg.NLOC]
        m["x"] = xs
        in_maps.append(m)
    nc = build(cfg, meta, float(np.asarray(weights["lin1_b"]).reshape(-1)[0]))
    res = run_bass_kernel_spmd(
        nc, in_maps, core_ids=list(range(NCORES)), trace=trace
    )
    out = np.concatenate(
        [res.results[c]["out"][: cfg.NLOC] for c in range(NCORES)], axis=0
    )
    return out, res


def kernel(**inputs) -> np.ndarray:
    cfg = Cfg(N=100000)
    weights = {
        k: np.asarray(v)
        for k, v in inputs.items()
        if k not in ("x", "edge_index")
    }
    # reference names: W0,b0,W1,b1,gn0_*,gn1_*,lin0_*,lin1_*
    wmap = dict(weights)
    wmap["b0"] = weights["b0"]
    wmap["b1"] = weights["b1"]
    out, _ = run(cfg, np.asarray(inputs["x"]), np.asarray(inputs["edge_index"]), wmap)
    return out.astype(np.float32)


# revision 13
# speedup vs baseline: 1.1289x; 1.1289x over previous
"""GCN (2x GCNConv + GraphNorm + ReLU, MLP head) on 8 TRN2 NeuronCores.

Sharding: destination-node ranges across the 8 cores. Per layer each core
computes its shard of h = (dinv * x) @ W (bf16), AllGathers the full node
table into DRAM, DMA-gathers the source rows of its own (dest-sorted,
source-quadrant-bucketed) edges and segment-sums them with one-hot matmuls
on the TensorEngine, accumulating 128-dest windows in PSUM. Self-loop
contributions are folded into the PSUM drain from the locally staged table.
GraphNorm statistics go through small AllReduces. All data-dependent
structure (gather indices, one-hot columns) is carried by input tensors so
a single program serves all 8 cores.
"""

from dataclasses import dataclass, field

import ml_dtypes
import numpy as np

import concourse.bacc as bacc
import concourse.bass as bass
import concourse.mybir as mybir
import concourse.tile as tile
from concourse.bass_utils import run_bass_kernel_spmd

F32 = mybir.dt.float32
BF16 = mybir.dt.bfloat16
I16 = mybir.dt.int16

AF = mybir.ActivationFunctionType
ALU = mybir.AluOpType

NCORES = 8
NQUAD = 4
D = 128
EPS = 1e-5
ACT_EVERY = 5  # every ACT_EVERY-th one-hot build goes to ScalarE instead of DVE


@dataclass
class Cfg:
    N: int = 100000
    CH: int = 8  # gather chunk, in 128-edge tiles (num_idxs<=1024 single packet)
    NLOC: int = field(init=False)
    NLOC_PAD: int = field(init=False)
    W: int = field(init=False)
    QROWS: int = field(init=False)
    TROWS: int = field(init=False)

    def __post_init__(self):
        assert self.N % NCORES == 0
        self.NLOC = self.N // NCORES
        self.W = (self.NLOC + 127) // 128
        self.NLOC_PAD = self.W * 128
        self.QROWS = (NCORES // NQUAD) * self.NLOC_PAD
        self.TROWS = NCORES * self.NLOC_PAD
        assert self.QROWS <= 32768


def preprocess(cfg: Cfg, edge_index: np.ndarray):
    """64-slot block scheme: per (bucket, window) groups padded to 64-slot
    blocks; 128-edge gather tiles = block pairs; straddling tiles get one
    matmul slot per touched window. Self-loops are excluded (folded into the
    drain on-device)."""
    N, NLOC, NLOC_PAD, W = cfg.N, cfg.NLOC, cfg.NLOC_PAD, cfg.W
    row = edge_index[0].astype(np.int64)
    col = edge_index[1].astype(np.int64)

    deg = (np.bincount(col, minlength=N) + 1).astype(np.float64)  # + self loop
    dinv = (1.0 / np.sqrt(deg)).astype(np.float32)

    src_core = row // NLOC
    trow = src_core * NLOC_PAD + (row - src_core * NLOC)
    quad = trow // cfg.QROWS
    qidx = (trow - quad * cfg.QROWS).astype(np.int16)
    dest_core = col // NLOC
    ld = col - dest_core * NLOC
    win = ld // 128
    doff_all = (ld - win * 128).astype(np.float32)

    cnt = np.zeros((NCORES, NQUAD, W), dtype=np.int64)
    np.add.at(cnt, (dest_core, quad, win), 1)

    K64 = np.ceil(cnt / 64.0).astype(np.int64).max(axis=0)  # [NQUAD, W]
    assert (K64.sum(axis=0) > 0).all()

    # block/tile structure per bucket (shared across cores)
    block_wins = []
    T_b = []
    for b in range(NQUAD):
        bw = []
        for w in range(W):
            bw += [w] * int(K64[b, w])
        if len(bw) % 2:
            bw.append(-1)
        block_wins.append(bw)
        T_b.append(len(bw) // 2)
    T_b = np.array(T_b, dtype=np.int64)
    CH = cfg.CH
    T_b_pad = ((T_b + CH - 1) // CH) * CH

    # matmul slot schedule: windows ascending; per window, buckets; per
    # bucket the tiles touching that window in order. half: 0 = block A
    # valid only, 1 = block B only, 2 = both blocks in this window.
    slots_by_w = [[] for _ in range(W)]
    for b in range(NQUAD):
        bw = block_wins[b]
        for t in range(int(T_b[b])):
            wa, wb = bw[2 * t], bw[2 * t + 1]
            if wa == wb:
                slots_by_w[wa].append((b, t, 2))
            else:
                if wa >= 0:
                    slots_by_w[wa].append((b, t, 0))
                if wb >= 0:
                    slots_by_w[wb].append((b, t, 1))
    sched = []  # flat: (w, b, t, half)
    slots_per_w = []
    for w in range(W):
        slots_per_w.append(len(slots_by_w[w]))
        for (b, t, half) in slots_by_w[w]:
            sched.append((w, b, t, half))
    T2 = len(sched)

    # map block position -> (window, k-th block of that (b,w) group)
    blk_k = {}
    for b in range(NQUAD):
        kc = {}
        for i, w in enumerate(block_wins[b]):
            if w < 0:
                blk_k[(b, i)] = None
                continue
            k = kc.get(w, 0)
            kc[w] = k + 1
            blk_k[(b, i)] = (w, k)

    ins = []
    for c in range(NCORES):
        m = dest_core == c
        q_c, w_c = quad[m], win[m]
        order = np.argsort(q_c * W + w_c, kind="stable")
        qi_c = qidx[m][order]
        do_c = doff_all[m][order]
        starts = np.zeros((NQUAD, W + 1), dtype=np.int64)
        for b in range(NQUAD):
            for w in range(W):
                starts[b, w + 1] = starts[b, w] + cnt[c, b, w]
        base_b = np.concatenate([[0], np.cumsum(starts[:, -1])])

        blk_idx = {}
        blk_doff = {}
        for b in range(NQUAD):
            for w in range(W):
                lo = base_b[b] + starts[b, w]
                n = int(cnt[c, b, w])
                nb = int(K64[b, w])
                ibuf = np.zeros(nb * 64, np.int16)
                dbuf = np.full(nb * 64, -1.0, np.float32)
                ibuf[:n] = qi_c[lo : lo + n]
                dbuf[:n] = do_c[lo : lo + n]
                for k in range(nb):
                    blk_idx[(b, w, k)] = ibuf[64 * k : 64 * (k + 1)]
                    blk_doff[(b, w, k)] = dbuf[64 * k : 64 * (k + 1)]

        core_in = {}
        for b in range(NQUAD):
            bw = block_wins[b]
            stream = np.zeros(int(T_b_pad[b]) * 128, np.int16)
            for i in range(len(bw)):
                bk = blk_k[(b, i)]
                if bk is None:
                    continue
                stream[i * 64 : (i + 1) * 64] = blk_idx[(b, bk[0], bk[1])]
            wrapped = stream.reshape(-1, 16).T
            core_in[f"idx{b}"] = np.tile(wrapped, (8, 1)).copy()

        doff_slots = np.full((T2, 128), -1.0, np.float32)
        for s, (w, b, t, half) in enumerate(sched):
            dv = np.full(128, -1.0, np.float32)
            if half in (0, 2):
                bk = blk_k[(b, 2 * t)]
                if bk is not None:
                    dv[:64] = blk_doff[(b, bk[0], bk[1])]
            if half in (1, 2):
                bk = blk_k[(b, 2 * t + 1)]
                if bk is not None:
                    dv[64:] = blk_doff[(b, bk[0], bk[1])]
            doff_slots[s] = dv
        core_in["doff"] = doff_slots.T.copy()
        core_in["doffn"] = (-doff_slots.T).copy()

        dl = np.zeros(NLOC_PAD, np.float32)
        dl[:NLOC] = dinv[c * NLOC : (c + 1) * NLOC]
        core_in["dinvw"] = dl.reshape(W, 128).T.copy()
        core_in["dinv2w"] = (core_in["dinvw"] ** 2).copy()
        mk = np.zeros(NLOC_PAD, np.float32)
        mk[:NLOC] = 1.0
        core_in["maskw"] = mk.reshape(W, 128).T.copy()
        ins.append(core_in)

    meta = dict(
        K64=K64,
        T_b=T_b,
        T_b_pad=T_b_pad,
        T2=T2,
        sched=sched,
        slots_per_w=slots_per_w,
        dinv=dinv,
    )
    return ins, meta


def build(cfg: Cfg, meta, lin1b: float) -> bacc.Bacc:
    N, NLOC_PAD, W, CH = cfg.N, cfg.NLOC_PAD, cfg.W, cfg.CH
    T_b, T_b_pad, T2 = meta["T_b"], meta["T_b_pad"], meta["T2"]
    sched, slots_per_w = meta["sched"], meta["slots_per_w"]

    nc = bacc.Bacc(
        "TRN2",
        target_bir_lowering=False,
        debug=False,
        num_devices=NCORES,
        num_swdge_queues=4,
    )

    X = nc.dram_tensor("x", [NLOC_PAD, D], F32, kind="ExternalInput")
    IDX = [
        nc.dram_tensor(f"idx{b}", [128, int(T_b_pad[b]) * 8], I16, kind="ExternalInput")
        for b in range(NQUAD)
    ]
    DOFF = nc.dram_tensor("doff", [128, T2], F32, kind="ExternalInput")
    DOFFN = nc.dram_tensor("doffn", [128, T2], F32, kind="ExternalInput")
    DINVW = nc.dram_tensor("dinvw", [128, W], F32, kind="ExternalInput")
    DINV2W = nc.dram_tensor("dinv2w", [128, W], F32, kind="ExternalInput")
    MASKW = nc.dram_tensor("maskw", [128, W], F32, kind="ExternalInput")
    IOTA = nc.dram_tensor("iota", [128, 128], BF16, kind="ExternalInput")
    IDENT = nc.dram_tensor("ident", [128, 128], F32, kind="ExternalInput")
    ONESROW = nc.dram_tensor("onesrow", [1, 128], F32, kind="ExternalInput")
    ONESCOL = nc.dram_tensor("onescol", [128, 1], F32, kind="ExternalInput")
    WMAT = [nc.dram_tensor(f"w{l}", [D, D], F32, kind="ExternalInput") for l in range(2)]
    GN_A = [nc.dram_tensor(f"gn{l}_a", [D, 1], F32, kind="ExternalInput") for l in range(2)]
    GN_W = [nc.dram_tensor(f"gn{l}_w", [D, 1], F32, kind="ExternalInput") for l in range(2)]
    GN_B = [nc.dram_tensor(f"gn{l}_b", [D, 1], F32, kind="ExternalInput") for l in range(2)]
    BCONV = [nc.dram_tensor(f"b{l}", [D, 1], F32, kind="ExternalInput") for l in range(2)]
    LIN0 = nc.dram_tensor("lin0_w", [D, D], F32, kind="ExternalInput")
    LIN0B = nc.dram_tensor("lin0_b", [D, 1], F32, kind="ExternalInput")
    LIN1 = nc.dram_tensor("lin1_w", [D, 1], F32, kind="ExternalInput")
    OUT = nc.dram_tensor("out", [NLOC_PAD, 1], F32, kind="ExternalOutput")

    SHARD = nc.dram_tensor("shard", [NLOC_PAD, D], BF16)
    TABLE = nc.dram_tensor("table", [cfg.TROWS, D], BF16, addr_space="Shared")
    RS_IN = nc.dram_tensor("rs_in", [D, 1], F32)
    RS_OUT = nc.dram_tensor("rs_out", [D, 1], F32, addr_space="Shared")
    RS_IN2 = nc.dram_tensor("rs_in2", [D, 1], F32)
    RS_OUT2 = nc.dram_tensor("rs_out2", [D, 1], F32, addr_space="Shared")

    rg = [list(range(NCORES))]

    with tile.TileContext(nc) as tc:
        import contextlib

        ctx = contextlib.ExitStack()
        with ctx:
            sb = ctx.enter_context(tc.tile_pool(name="sb", bufs=1))
            x_sb = sb.tile([128, W * D], F32, tag="x", name="x_sb")
            stage = sb.tile([128, W * D], BF16, tag="stage", name="stage")
            doff_sb = sb.tile([128, T2], F32, tag="doff", name="doff_sb")
            doffn_sb = sb.tile([128, T2], F32, tag="doffn", name="doffn_sb")
            dinv_sb = sb.tile([128, W], F32, tag="dinv", name="dinv_sb")
            dinv2_sb = sb.tile([128, W], F32, tag="dinv2", name="dinv2_sb")
            mask_sb = sb.tile([128, W], F32, tag="mask", name="mask_sb")
            iota_sb = sb.tile([128, 128], BF16, tag="iota", name="iota_sb")
            ident_sb = sb.tile([128, 128], F32, tag="ident", name="ident_sb")
            onesrow_sb = sb.tile([1, 128], F32, tag="onesrow", name="onesrow_sb")
            onescol_sb = sb.tile([128, 1], F32, tag="onescol", name="onescol_sb")
            w_sb = [sb.tile([D, D], F32, tag=f"w{l}", name=f"w{l}_sb") for l in range(2)]
            gna_sb = [sb.tile([D, 1], F32, tag=f"gna{l}", name=f"gna{l}_sb") for l in range(2)]
            gnw_sb = [sb.tile([D, 1], F32, tag=f"gnw{l}", name=f"gnw{l}_sb") for l in range(2)]
            gnb_sb = [sb.tile([D, 1], F32, tag=f"gnb{l}", name=f"gnb{l}_sb") for l in range(2)]
            bconv_sb = [sb.tile([D, 1], F32, tag=f"bc{l}", name=f"bc{l}_sb") for l in range(2)]
            lin0_sb = sb.tile([D, D], F32, tag="lin0", name="lin0_sb")
            lin0b_sb = sb.tile([D, 1], F32, tag="lin0b", name="lin0b_sb")
            lin1_sb = sb.tile([D, 1], F32, tag="lin1", name="lin1_sb")
            idx_sb = [
                sb.tile([128, int(T_b_pad[b]) * 8], I16, tag=f"idx{b}", name=f"idx{b}_sb")
                for b in range(NQUAD)
            ]
            out_sb = sb.tile([128, W], F32, tag="outsb", name="out_sb")

            nc.sync.dma_start(
                x_sb[:].rearrange("p (w d) -> p w d", w=W),
                X.ap().rearrange("(w p) d -> p w d", p=128),
            )
            nc.sync.dma_start(doff_sb[:], DOFF[:])
            nc.sync.dma_start(doffn_sb[:], DOFFN[:])
            nc.sync.dma_start(dinv_sb[:], DINVW[:])
            nc.sync.dma_start(dinv2_sb[:], DINV2W[:])
            nc.sync.dma_start(mask_sb[:], MASKW[:])
            nc.sync.dma_start(iota_sb[:], IOTA[:])
            nc.sync.dma_start(ident_sb[:], IDENT[:])
            nc.sync.dma_start(onesrow_sb[:], ONESROW[:])
            nc.sync.dma_start(onescol_sb[:], ONESCOL[:])
            for l in range(2):
                nc.sync.dma_start(w_sb[l][:], WMAT[l][:])
                nc.sync.dma_start(gna_sb[l][:], GN_A[l][:])
                nc.sync.dma_start(gnw_sb[l][:], GN_W[l][:])
                nc.sync.dma_start(gnb_sb[l][:], GN_B[l][:])
                nc.sync.dma_start(bconv_sb[l][:], BCONV[l][:])
            nc.sync.dma_start(lin0_sb[:], LIN0[:])
            nc.sync.dma_start(lin0b_sb[:], LIN0B[:])
            nc.sync.dma_start(lin1_sb[:], LIN1[:])
            for b in range(NQUAD):
                nc.sync.dma_start(idx_sb[b][:], IDX[b][:])

            ps_t = ctx.enter_context(tc.tile_pool(name="ps_t", bufs=2, space="PSUM"))
            ps_h = ctx.enter_context(tc.tile_pool(name="ps_h", bufs=2, space="PSUM"))
            ps_w = ctx.enter_context(tc.tile_pool(name="ps_w", bufs=3, space="PSUM"))
            ps_s = ctx.enter_context(tc.tile_pool(name="ps_s", bufs=1, space="PSUM"))
            sp = ctx.enter_context(tc.tile_pool(name="sp", bufs=4))
            gst = [
                ctx.enter_context(tc.tile_pool(name=f"g{b}", bufs=3))
                for b in range(NQUAD)
            ]

            def bcast_row(src_col, scratch_tag):
                tp = ps_t.tile([1, 128], F32, tag="tp", name="tp_row")
                nc.tensor.transpose(tp[:], src_col, ident_sb[:])
                rrow = sp.tile([1, 128], F32, tag=scratch_tag + "_row", name=scratch_tag + "_row")
                nc.scalar.activation(rrow[:], tp[:], AF.Copy)
                bc = ps_t.tile([128, 128], F32, tag="tp", name="tp_bc")
                nc.tensor.matmul(bc[:], onesrow_sb[:], rrow[:], start=True, stop=True)
                t = sp.tile([128, 128], F32, tag=scratch_tag, name=scratch_tag)
                nc.scalar.activation(t[:], bc[:], AF.Copy)
                return t

            def prologue(layer):
                for w in range(W):
                    xw = x_sb[:, w * D : (w + 1) * D]
                    xs = sp.tile([128, D], F32, tag="p_xs", name="p_xs")
                    nc.vector.tensor_scalar_mul(xs[:], xw, dinv_sb[:, w : w + 1])
                    tp = ps_t.tile([128, D], F32, tag="tp", name="p_tp")
                    nc.tensor.transpose(tp[:], xs[:], ident_sb[:])
                    xT = sp.tile([128, D], F32, tag="p_xT", name="p_xT")
                    nc.scalar.activation(xT[:], tp[:], AF.Copy)
                    hp = ps_h.tile([128, D], F32, tag="hp", name="p_hp")
                    nc.tensor.matmul(hp[:], xT[:], w_sb[layer][:], start=True, stop=True)
                    nc.scalar.activation(stage[:, w * D : (w + 1) * D], hp[:], AF.Copy)
                nc.sync.dma_start(
                    SHARD.ap().rearrange("(w p) d -> p w d", p=128),
                    stage[:].rearrange("p (w d) -> p w d", w=W),
                )
                nc.gpsimd.collective_compute(
                    "AllGather",
                    ALU.bypass,
                    replica_groups=rg,
                    ins=[SHARD.ap().opt()],
                    outs=[TABLE.ap().opt()],
                )

            def allreduce(col_ap, bounce_in, bounce_out, tag):
                t = sp.tile([D, 1], F32, tag=tag, name=tag + "_t")
                nc.scalar.activation(t[:], col_ap, AF.Copy)
                nc.sync.dma_start(bounce_in[:], t[:])
                nc.gpsimd.collective_compute(
                    "AllReduce",
                    ALU.add,
                    replica_groups=rg,
                    ins=[bounce_in.ap().opt()],
                    outs=[bounce_out.ap().opt()],
                )
                r = sp.tile([D, 1], F32, tag=tag + "_r", name=tag + "_r")
                nc.sync.dma_start(r[:], bounce_out[:])
                return r

            def gather_and_aggregate(layer):
                chunk_tiles = [dict() for _ in range(NQUAD)]
                s = 0
                for w in range(W):
                    nslots = slots_per_w[w]
                    pw = ps_w.tile([128, D], F32, tag="agg", name="agg_pw")
                    for si in range(nslots):
                        (w_, b, t, half) = sched[s]
                        assert w_ == w
                        cidx = t // CH
                        if cidx not in chunk_tiles[b]:
                            g = gst[b].tile([128, CH, D], BF16, tag="g", name=f"g{b}_t")
                            nidx = CH * 128
                            nc.gpsimd.dma_gather(
                                g[:],
                                TABLE.ap()[b * cfg.QROWS : (b + 1) * cfg.QROWS, :],
                                idx_sb[b][:, cidx * CH * 8 : (cidx + 1) * CH * 8],
                                nidx,
                                nidx,
                                D,
                                queue_num=b,
                            )
                            chunk_tiles[b] = {cidx: g}
                        g = chunk_tiles[b][cidx]
                        s_t = sp.tile([128, 128], BF16, tag="s_onehot", name="s_onehot")
                        if s % ACT_EVERY == ACT_EVERY - 1:
                            u = sp.tile([128, 128], BF16, tag="s_u", name="s_u")
                            nc.scalar.activation(
                                u[:], iota_sb[:], AF.Square,
                                bias=doffn_sb[:, s : s + 1],
                            )
                            nc.scalar.activation(
                                s_t[:], u[:], AF.Relu, bias=1.0, scale=-1.0
                            )
                        else:
                            nc.vector.tensor_scalar(
                                s_t[:],
                                iota_sb[:],
                                doff_sb[:, s : s + 1],
                                None,
                                op0=ALU.is_equal,
                            )
                        nc.tensor.matmul(
                            pw[:],
                            s_t[:],
                            g[:, t % CH, :],
                            start=(si == 0),
                            stop=(si == nslots - 1),
                        )
                        s += 1
                    # drain: x = psum*dinv + stage*dinv^2 (self loop folded)
                    ss = sp.tile([128, D], F32, tag="ss", name="ss")
                    nc.vector.tensor_scalar_mul(
                        ss[:], stage[:, w * D : (w + 1) * D], dinv_sb[:, w : w + 1]
                    )
                    nc.vector.scalar_tensor_tensor(
                        x_sb[:, w * D : (w + 1) * D],
                        pw[:],
                        dinv_sb[:, w : w + 1],
                        ss[:],
                        op0=ALU.mult,
                        op1=ALU.add,
                    )
                assert s == T2

            def graphnorm_relu(layer):
                sps = ps_s.tile([128, 1], F32, tag="stats", name="stats_ps")
                for w in range(W):
                    nc.tensor.matmul(
                        sps[:],
                        x_sb[:, w * D : (w + 1) * D],
                        onescol_sb[:],
                        start=(w == 0),
                        stop=(w == W - 1),
                    )
                gsum = allreduce(sps[:], RS_IN, RS_OUT, "ar_mean")
                m2 = sp.tile([D, 1], F32, tag="m2", name="m2")
                nc.vector.tensor_scalar(m2[:], gsum[:], 1.0 / N, None, op0=ALU.mult)
                nc.vector.tensor_add(m2[:], m2[:], bconv_sb[layer][:])
                nc.vector.tensor_mul(m2[:], m2[:], gna_sb[layer][:])
                nc.vector.tensor_sub(m2[:], m2[:], bconv_sb[layer][:])
                m2t = bcast_row(m2[:], "m2bc")
                vps = ps_s.tile([128, 1], F32, tag="stats", name="stats2_ps")
                for w in range(W):
                    xw = x_sb[:, w * D : (w + 1) * D]
                    nc.vector.tensor_sub(xw, xw, m2t[:])
                    nc.vector.tensor_scalar_mul(xw, xw, mask_sb[:, w : w + 1])
                    sq = sp.tile([128, D], F32, tag="sq", name="sq")
                    nc.vector.tensor_mul(sq[:], xw, xw)
                    nc.tensor.matmul(
                        vps[:],
                        sq[:],
                        onescol_sb[:],
                        start=(w == 0),
                        stop=(w == W - 1),
                    )
                gvar = allreduce(vps[:], RS_IN2, RS_OUT2, "ar_var")
                vs = sp.tile([D, 1], F32, tag="vs", name="vs")
                nc.vector.tensor_scalar(
                    vs[:], gvar[:], 1.0 / N, EPS, op0=ALU.mult, op1=ALU.add
                )
                rc = sp.tile([D, 1], F32, tag="rc", name="rc")
                nc.vector.reciprocal(rc[:], vs[:])
                rstd = sp.tile([D, 1], F32, tag="rstd", name="rstd")
                nc.scalar.activation(rstd[:], rc[:], AF.Sqrt)
                f = sp.tile([D, 1], F32, tag="fcol", name="fcol")
                nc.vector.tensor_mul(f[:], rstd[:], gnw_sb[layer][:])
                ft = bcast_row(f[:], "fbc")
                gt = bcast_row(gnb_sb[layer][:], "gbc")
                for w in range(W):
                    xw = x_sb[:, w * D : (w + 1) * D]
                    nc.vector.tensor_mul(xw, xw, ft[:])
                    nc.vector.tensor_add(xw, xw, gt[:])
                    nc.scalar.activation(xw, xw, AF.Relu)

            def mlp_head():
                b0t = bcast_row(lin0b_sb[:], "l0bc")
                for w in range(W):
                    xw = x_sb[:, w * D : (w + 1) * D]
                    tp = ps_t.tile([128, D], F32, tag="tp", name="m_tp")
                    nc.tensor.transpose(tp[:], xw, ident_sb[:])
                    xT = sp.tile([128, D], F32, tag="m_xT", name="m_xT")
                    nc.scalar.activation(xT[:], tp[:], AF.Copy)
                    yp = ps_h.tile([128, D], F32, tag="hp", name="m_yp")
                    nc.tensor.matmul(yp[:], xT[:], lin0_sb[:], start=True, stop=True)
                    y = sp.tile([128, D], F32, tag="m_y", name="m_y")
                    nc.vector.tensor_add(y[:], yp[:], b0t[:])
                    nc.scalar.activation(y[:], y[:], AF.Relu)
                    tp2 = ps_t.tile([128, D], F32, tag="tp", name="m_tp2")
                    nc.tensor.transpose(tp2[:], y[:], ident_sb[:])
                    yT = sp.tile([128, D], F32, tag="m_yT", name="m_yT")
                    nc.scalar.activation(yT[:], tp2[:], AF.Copy)
                    op = ps_h.tile([128, 1], F32, tag="hp", name="m_op")
                    nc.tensor.matmul(op[:], yT[:], lin1_sb[:], start=True, stop=True)
                    nc.vector.tensor_scalar_add(out_sb[:, w : w + 1], op[:], lin1b)
                nc.sync.dma_start(
                    OUT.ap().rearrange("(w p) one -> p w one", p=128),
                    out_sb[:].rearrange("p (w one) -> p w one", one=1),
                )

            for layer in range(2):
                prologue(layer)
                gather_and_aggregate(layer)
                graphnorm_relu(layer)
            mlp_head()

    nc.compile()
    return nc


def _make_const_inputs(cfg: Cfg, weights: dict):
    c = {}
    c["iota"] = np.broadcast_to(
        np.arange(128, dtype=np.float32), (128, 128)
    ).astype(ml_dtypes.bfloat16)
    c["ident"] = np.eye(128, dtype=np.float32)
    c["onesrow"] = np.ones((1, 128), np.float32)
    c["onescol"] = np.ones((128, 1), np.float32)
    c["w0"] = np.asarray(weights["W0"], np.float32)
    c["w1"] = np.asarray(weights["W1"], np.float32)
    for l in range(2):
        c[f"gn{l}_a"] = np.asarray(weights[f"gn{l}_a"], np.float32).reshape(D, 1)
        c[f"gn{l}_w"] = np.asarray(weights[f"gn{l}_w"], np.float32).reshape(D, 1)
        c[f"gn{l}_b"] = np.asarray(weights[f"gn{l}_b"], np.float32).reshape(D, 1)
        c[f"b{l}"] = np.asarray(weights[f"b{l}"], np.float32).reshape(D, 1)
    c["lin0_w"] = np.asarray(weights["lin0_w"], np.float32)
    c["lin0_b"] = np.asarray(weights["lin0_b"], np.float32).reshape(D, 1)
    c["lin1_w"] = np.asarray(weights["lin1_w"], np.float32).reshape(D, 1)
    return c


def run(cfg: Cfg, x, edge_index, weights, trace=False):
    ins, meta = preprocess(cfg, edge_index)
    consts = _make_const_inputs(cfg, weights)
    x = np.asarray(x, np.float32)
    in_maps = []
    for c in range(NCORES):
        m = dict(ins[c])
        m.update(consts)
        xs = np.zeros((cfg.NLOC_PAD, D), np.float32)
        xs[: cfg.NLOC] = x[c * cfg.NLOC : (c + 1) * cfg.NLOC]
        m["x"] = xs
        in_maps.append(m)
    nc = build(cfg, meta, float(np.asarray(weights["lin1_b"]).reshape(-1)[0]))
    res = run_bass_kernel_spmd(nc, in_maps, core_ids=list(range(NCORES)), trace=trace)
    out = np.concatenate(
        [res.results[c]["out"][: cfg.NLOC] for c in range(NCORES)], axis=0
    )
    return out, res


def kernel(**inputs) -> np.ndarray:
    cfg = Cfg(N=100000)
    weights = {
        k: np.asarray(v) for k, v in inputs.items() if k not in ("x", "edge_index")
    }
    out, _ = run(
        cfg, np.asarray(inputs["x"]), np.asarray(inputs["edge_index"]), weights
    )
    return out.astype(np.float32)


# revision 21
# speedup vs baseline: 1.2421x; 1.1003x over previous
"""GCN (2x GCNConv + GraphNorm + ReLU, MLP head) on 8 TRN2 NeuronCores.

Sharding: destination-node ranges across the 8 cores, feature-transposed
canonical layout x^T [D, nodes]. Per layer each core computes its shard of
h = (dinv * x) @ W (bf16, via one W-stationary matmul sweep), PE-transposes
it into row-major form, AllGathers the full node table into DRAM, then
DMA-gathers the source rows of its own (dest-sorted, source-quadrant
bucketed) edges. Segment-sum runs on the TensorEngine: per 128-edge tile,
out^T[D, dests] += G^T @ S with host-precomputed one-hot S tiles streamed
from DRAM (no on-device one-hot builds), accumulating 128-dest windows in
PSUM. Self-loops are folded into the PSUM drain from the locally staged
table. GraphNorm statistics are single DVE reductions plus tiny AllReduces.
All data-dependent structure (gather indices, one-hot S) is carried by
input tensors so a single program serves all 8 cores.
"""

from dataclasses import dataclass, field

import ml_dtypes
import numpy as np

import concourse.bacc as bacc
import concourse.bass as bass
import concourse.mybir as mybir
import concourse.tile as tile
from concourse.bass_utils import run_bass_kernel_spmd

F32 = mybir.dt.float32
BF16 = mybir.dt.bfloat16
I16 = mybir.dt.int16

AF = mybir.ActivationFunctionType
ALU = mybir.AluOpType
AXIS = mybir.AxisListType

NCORES = 8
NQUAD = 4
D = 128
EPS = 1e-5


@dataclass
class Cfg:
    N: int = 100000
    CH: int = 8  # gather chunk, in 128-edge tiles (num_idxs<=1024 single packet)
    SCH: int = 16  # S-matrix DMA chunk, in slots
    MMCH: int = 512  # prologue/mlp matmul free-dim chunk
    NLOC: int = field(init=False)
    NLOC_PAD: int = field(init=False)
    W: int = field(init=False)
    QROWS: int = field(init=False)
    TROWS: int = field(init=False)

    def __post_init__(self):
        assert self.N % NCORES == 0
        self.NLOC = self.N // NCORES
        self.W = (self.NLOC + 127) // 128
        self.NLOC_PAD = self.W * 128
        self.QROWS = (NCORES // NQUAD) * self.NLOC_PAD
        self.TROWS = NCORES * self.NLOC_PAD
        assert self.QROWS <= 32768
        self.MMCH = min(self.MMCH, self.NLOC_PAD)
        while self.NLOC_PAD % self.MMCH:
            self.MMCH -= 64
        assert self.MMCH > 0 and self.NLOC_PAD % self.MMCH == 0


def preprocess(cfg: Cfg, edge_index: np.ndarray):
    """64-slot block scheme: per (bucket, window) groups padded to 64-slot
    blocks; 128-edge gather tiles = block pairs; straddling tiles get one
    matmul slot per touched window. Self-loops excluded (folded into drain).
    One-hot S tiles [T2, 128 edge, 128 dest] are precomputed per core."""
    N, NLOC, NLOC_PAD, W = cfg.N, cfg.NLOC, cfg.NLOC_PAD, cfg.W
    row = edge_index[0].astype(np.int64)
    col = edge_index[1].astype(np.int64)

    deg = (np.bincount(col, minlength=N) + 1).astype(np.float64)  # + self loop
    dinv = (1.0 / np.sqrt(deg)).astype(np.float32)

    src_core = row // NLOC
    trow = src_core * NLOC_PAD + (row - src_core * NLOC)
    quad = trow // cfg.QROWS
    qidx = (trow - quad * cfg.QROWS).astype(np.int16)
    dest_core = col // NLOC
    ld = col - dest_core * NLOC
    win = ld // 128
    doff_all = (ld - win * 128).astype(np.int64)

    cnt = np.zeros((NCORES, NQUAD, W), dtype=np.int64)
    np.add.at(cnt, (dest_core, quad, win), 1)

    K64 = np.ceil(cnt / 64.0).astype(np.int64).max(axis=0)  # [NQUAD, W]
    assert (K64.sum(axis=0) > 0).all()

    block_wins = []
    T_b = []
    for b in range(NQUAD):
        bw = []
        for w in range(W):
            bw += [w] * int(K64[b, w])
        if len(bw) % 2:
            bw.append(-1)
        block_wins.append(bw)
        T_b.append(len(bw) // 2)
    T_b = np.array(T_b, dtype=np.int64)
    CH = cfg.CH
    T_b_pad = ((T_b + CH - 1) // CH) * CH

    slots_by_w = [[] for _ in range(W)]
    for b in range(NQUAD):
        bw = block_wins[b]
        for t in range(int(T_b[b])):
            wa, wb = bw[2 * t], bw[2 * t + 1]
            if wa == wb:
                slots_by_w[wa].append((b, t, 2))
            else:
                if wa >= 0:
                    slots_by_w[wa].append((b, t, 0))
                if wb >= 0:
                    slots_by_w[wb].append((b, t, 1))
    sched = []
    slots_per_w = []
    for w in range(W):
        slots_per_w.append(len(slots_by_w[w]))
        for (b, t, half) in slots_by_w[w]:
            sched.append((w, b, t, half))
    T2 = len(sched)

    blk_k = {}
    for b in range(NQUAD):
        kc = {}
        for i, w in enumerate(block_wins[b]):
            if w < 0:
                blk_k[(b, i)] = None
                continue
            k = kc.get(w, 0)
            kc[w] = k + 1
            blk_k[(b, i)] = (w, k)

    ins = []
    for c in range(NCORES):
        m = dest_core == c
        q_c, w_c = quad[m], win[m]
        order = np.argsort(q_c * W + w_c, kind="stable")
        qi_c = qidx[m][order]
        do_c = doff_all[m][order]
        starts = np.zeros((NQUAD, W + 1), dtype=np.int64)
        for b in range(NQUAD):
            for w in range(W):
                starts[b, w + 1] = starts[b, w] + cnt[c, b, w]
        base_b = np.concatenate([[0], np.cumsum(starts[:, -1])])

        blk_idx = {}
        blk_doff = {}
        for b in range(NQUAD):
            for w in range(W):
                lo = base_b[b] + starts[b, w]
                n = int(cnt[c, b, w])
                nb = int(K64[b, w])
                ibuf = np.zeros(nb * 64, np.int16)
                dbuf = np.full(nb * 64, -1, np.int64)
                ibuf[:n] = qi_c[lo : lo + n]
                dbuf[:n] = do_c[lo : lo + n]
                for k in range(nb):
                    blk_idx[(b, w, k)] = ibuf[64 * k : 64 * (k + 1)]
                    blk_doff[(b, w, k)] = dbuf[64 * k : 64 * (k + 1)]

        core_in = {}
        for b in range(NQUAD):
            bw = block_wins[b]
            stream = np.zeros(int(T_b_pad[b]) * 128, np.int16)
            for i in range(len(bw)):
                bk = blk_k[(b, i)]
                if bk is None:
                    continue
                stream[i * 64 : (i + 1) * 64] = blk_idx[(b, bk[0], bk[1])]
            wrapped = stream.reshape(-1, 16).T
            core_in[f"idx{b}"] = np.tile(wrapped, (8, 1)).copy()

        doff_slots = np.full((T2, 128), -1, np.int64)
        for s, (w, b, t, half) in enumerate(sched):
            dv = np.full(128, -1, np.int64)
            if half in (0, 2):
                bk = blk_k[(b, 2 * t)]
                if bk is not None:
                    dv[:64] = blk_doff[(b, bk[0], bk[1])]
            if half in (1, 2):
                bk = blk_k[(b, 2 * t + 1)]
                if bk is not None:
                    dv[64:] = blk_doff[(b, bk[0], bk[1])]
            doff_slots[s] = dv
        T2S = ((T2 + cfg.SCH - 1) // cfg.SCH) * cfg.SCH
        smat = np.zeros((T2S, 128, 128), dtype=ml_dtypes.bfloat16)
        si, ei = np.nonzero(doff_slots >= 0)
        smat[si, ei, doff_slots[si, ei]] = 1.0
        core_in["smat"] = smat

        dl = np.zeros(NLOC_PAD, np.float32)
        dl[:NLOC] = dinv[c * NLOC : (c + 1) * NLOC]
        core_in["dinvbc"] = np.broadcast_to(dl, (128, NLOC_PAD)).astype(
            ml_dtypes.bfloat16
        )
        ins.append(core_in)

    meta = dict(
        K64=K64, T_b=T_b, T_b_pad=T_b_pad, T2=T2,
        sched=sched, slots_per_w=slots_per_w, dinv=dinv,
    )
    return ins, meta


def build(cfg: Cfg, meta, lin1b: float) -> bacc.Bacc:
    N, NLOC_PAD, W, CH, SCH = cfg.N, cfg.NLOC_PAD, cfg.W, cfg.CH, cfg.SCH
    MMCH = cfg.MMCH
    T_b, T_b_pad, T2 = meta["T_b"], meta["T_b_pad"], meta["T2"]
    sched, slots_per_w = meta["sched"], meta["slots_per_w"]
    NMM = NLOC_PAD // MMCH

    nc = bacc.Bacc(
        "TRN2", target_bir_lowering=False, debug=False,
        num_devices=NCORES, num_swdge_queues=4,
    )

    XT = nc.dram_tensor("xt", [D, NLOC_PAD], F32, kind="ExternalInput")
    IDX = [
        nc.dram_tensor(f"idx{b}", [128, int(T_b_pad[b]) * 8], I16, kind="ExternalInput")
        for b in range(NQUAD)
    ]
    T2S = ((T2 + SCH - 1) // SCH) * SCH
    SMAT = nc.dram_tensor("smat", [T2S, 128, 128], BF16, kind="ExternalInput")
    DINVBC = nc.dram_tensor("dinvbc", [128, NLOC_PAD], BF16, kind="ExternalInput")
    IDENTB = nc.dram_tensor("identb", [128, 128], BF16, kind="ExternalInput")
    WMAT = [nc.dram_tensor(f"w{l}", [D, D], F32, kind="ExternalInput") for l in range(2)]
    GN_A = [nc.dram_tensor(f"gn{l}_a", [D, 1], F32, kind="ExternalInput") for l in range(2)]
    GN_W = [nc.dram_tensor(f"gn{l}_w", [D, 1], F32, kind="ExternalInput") for l in range(2)]
    GN_B = [nc.dram_tensor(f"gn{l}_b", [D, 1], F32, kind="ExternalInput") for l in range(2)]
    BCONV = [nc.dram_tensor(f"b{l}", [D, 1], F32, kind="ExternalInput") for l in range(2)]
    LIN0 = nc.dram_tensor("lin0_w", [D, D], F32, kind="ExternalInput")
    LIN0B = nc.dram_tensor("lin0_b", [D, 1], F32, kind="ExternalInput")
    LIN1 = nc.dram_tensor("lin1_w", [D, 1], F32, kind="ExternalInput")
    OUT = nc.dram_tensor("out", [1, NLOC_PAD], F32, kind="ExternalOutput")

    SHARD = nc.dram_tensor("shard", [NLOC_PAD, D], BF16)
    TABLE = nc.dram_tensor("table", [cfg.TROWS, D], BF16, addr_space="Shared")
    RS_IN = nc.dram_tensor("rs_in", [D, 1], F32)
    RS_OUT = nc.dram_tensor("rs_out", [D, 1], F32, addr_space="Shared")
    RS_IN2 = nc.dram_tensor("rs_in2", [D, 1], F32)
    RS_OUT2 = nc.dram_tensor("rs_out2", [D, 1], F32, addr_space="Shared")

    rg = [list(range(NCORES))]

    with tile.TileContext(nc) as tc:
        import contextlib

        ctx = contextlib.ExitStack()
        with ctx:
            sb = ctx.enter_context(tc.tile_pool(name="sb", bufs=1))
            x_sb = sb.tile([128, NLOC_PAD], F32, tag="x", name="x_sb")
            stage = sb.tile([128, NLOC_PAD], BF16, tag="stage", name="stage")
            tstage = sb.tile([128, W * D], BF16, tag="tstage", name="tstage")
            dinvbc_sb = sb.tile([128, NLOC_PAD], BF16, tag="dinvbc", name="dinvbc_sb")
            identb_sb = sb.tile([128, 128], BF16, tag="identb", name="identb_sb")
            w_sb = [sb.tile([D, D], F32, tag=f"w{l}", name=f"w{l}_sb") for l in range(2)]
            gna_sb = [sb.tile([D, 1], F32, tag=f"gna{l}", name=f"gna{l}_sb") for l in range(2)]
            gnw_sb = [sb.tile([D, 1], F32, tag=f"gnw{l}", name=f"gnw{l}_sb") for l in range(2)]
            gnb_sb = [sb.tile([D, 1], F32, tag=f"gnb{l}", name=f"gnb{l}_sb") for l in range(2)]
            bconv_sb = [sb.tile([D, 1], F32, tag=f"bc{l}", name=f"bc{l}_sb") for l in range(2)]
            lin0_sb = sb.tile([D, D], F32, tag="lin0", name="lin0_sb")
            lin0b_sb = sb.tile([D, 1], F32, tag="lin0b", name="lin0b_sb")
            lin1_sb = sb.tile([D, 1], F32, tag="lin1", name="lin1_sb")


            nc.sync.dma_start(x_sb[:], XT[:])
            nc.sync.dma_start(dinvbc_sb[:], DINVBC[:])
            nc.sync.dma_start(identb_sb[:], IDENTB[:])
            for l in range(2):
                nc.sync.dma_start(w_sb[l][:], WMAT[l][:])
                nc.sync.dma_start(gna_sb[l][:], GN_A[l][:])
                nc.sync.dma_start(gnw_sb[l][:], GN_W[l][:])
                nc.sync.dma_start(gnb_sb[l][:], GN_B[l][:])
                nc.sync.dma_start(bconv_sb[l][:], BCONV[l][:])
            nc.sync.dma_start(lin0_sb[:], LIN0[:])
            nc.sync.dma_start(lin0b_sb[:], LIN0B[:])
            nc.sync.dma_start(lin1_sb[:], LIN1[:])

            ps_t = ctx.enter_context(tc.tile_pool(name="ps_t", bufs=2, space="PSUM"))
            ps_h = ctx.enter_context(tc.tile_pool(name="ps_h", bufs=2, space="PSUM"))
            ps_w = ctx.enter_context(tc.tile_pool(name="ps_w", bufs=4, space="PSUM"))
            sp = ctx.enter_context(tc.tile_pool(name="sp", bufs=4))
            spool = ctx.enter_context(tc.tile_pool(name="spool", bufs=2))
            ipool = [
                ctx.enter_context(tc.tile_pool(name=f"i{b}", bufs=3))
                for b in range(NQUAD)
            ]
            gst = [
                ctx.enter_context(tc.tile_pool(name=f"g{b}", bufs=3))
                for b in range(NQUAD)
            ]

            def prologue(layer):
                # stage = bf16((dinv*x) @ W)^T, via W-stationary matmul chunks
                for k in range(NMM):
                    sl = slice(k * MMCH, (k + 1) * MMCH)
                    xs = sp.tile([128, MMCH], F32, tag="p_xs", name="p_xs")
                    nc.vector.tensor_mul(xs[:], x_sb[:, sl], dinvbc_sb[:, sl])
                    hp = ps_h.tile([128, MMCH], F32, tag="hp", name="p_hp")
                    nc.tensor.matmul(hp[:], w_sb[layer][:], xs[:], start=True, stop=True)
                    nc.scalar.activation(stage[:, sl], hp[:], AF.Copy)
                # row-major table staging via PE transpose per 128-node chunk
                for w in range(W):
                    tp = ps_t.tile([128, D], BF16, tag="tp", name="p_tp")
                    nc.tensor.transpose(
                        tp[:], stage[:, w * D : (w + 1) * D], identb_sb[:]
                    )
                    nc.scalar.activation(tstage[:, w * D : (w + 1) * D], tp[:], AF.Copy)
                nc.sync.dma_start(
                    SHARD.ap().rearrange("(w p) d -> p w d", p=128),
                    tstage[:].rearrange("p (w d) -> p w d", w=W),
                )
                nc.gpsimd.collective_compute(
                    "AllGather", ALU.bypass, replica_groups=rg,
                    ins=[SHARD.ap().opt()], outs=[TABLE.ap().opt()],
                )

            def allreduce(col_sb, bounce_in, bounce_out, tag):
                nc.sync.dma_start(bounce_in[:], col_sb)
                nc.gpsimd.collective_compute(
                    "AllReduce", ALU.add, replica_groups=rg,
                    ins=[bounce_in.ap().opt()], outs=[bounce_out.ap().opt()],
                )
                r = sp.tile([D, 1], F32, tag=tag + "_r", name=tag + "_r")
                nc.sync.dma_start(r[:], bounce_out[:])
                return r

            def gather_and_aggregate(layer):
                chunk_tiles = [dict() for _ in range(NQUAD)]
                schunks = {}
                s = 0
                for w in range(W):
                    nslots = slots_per_w[w]
                    pw = ps_w.tile([128, D], F32, tag="agg", name="agg_pw")
                    for si in range(nslots):
                        (w_, b, t, half) = sched[s]
                        cidx = t // CH
                        if cidx not in chunk_tiles[b]:
                            it = ipool[b].tile(
                                [128, CH * 8], I16, tag="i", name=f"i{b}_t"
                            )
                            nc.sync.dma_start(
                                it[:], IDX[b][:, cidx * CH * 8 : (cidx + 1) * CH * 8]
                            )
                            g = gst[b].tile([128, CH, D], BF16, tag="g", name=f"g{b}_t")
                            nidx = CH * 128
                            nc.gpsimd.dma_gather(
                                g[:],
                                TABLE.ap()[b * cfg.QROWS : (b + 1) * cfg.QROWS, :],
                                it[:], nidx, nidx, D, queue_num=b,
                            )
                            chunk_tiles[b] = {cidx: g}
                        g = chunk_tiles[b][cidx]
                        scidx = s // SCH
                        if scidx not in schunks:
                            sc = spool.tile([128, SCH, 128], BF16, tag="sc", name="sc")
                            nc.sync.dma_start(
                                sc[:],
                                SMAT.ap()[scidx * SCH : (scidx + 1) * SCH, :, :]
                                .rearrange("s e m -> e s m"),
                            )
                            schunks = {scidx: sc}
                        sc = schunks[scidx]
                        nc.tensor.matmul(
                            pw[:],
                            g[:, t % CH, :],
                            sc[:, s % SCH, :],
                            start=(si == 0),
                            stop=(si == nslots - 1),
                        )
                        s += 1
                    # drain: x^T[:, win] = (psum + stage_win) * dinvbc_win
                    wsl = slice(w * D, (w + 1) * D)
                    nc.vector.tensor_add(x_sb[:, wsl], pw[:], stage[:, wsl])
                    nc.vector.tensor_mul(x_sb[:, wsl], x_sb[:, wsl], dinvbc_sb[:, wsl])
                assert s == T2

            def graphnorm_relu(layer):
                NL = cfg.NLOC
                scol = sp.tile([D, 1], F32, tag="scol", name="scol")
                nc.vector.tensor_reduce(
                    scol[:], x_sb[:, :NL], axis=AXIS.X, op=ALU.add
                )
                gsum = allreduce(scol[:], RS_IN, RS_OUT, "ar_mean")
                m2 = sp.tile([D, 1], F32, tag="m2", name="m2")
                nc.vector.tensor_scalar(m2[:], gsum[:], 1.0 / N, None, op0=ALU.mult)
                nc.vector.tensor_add(m2[:], m2[:], bconv_sb[layer][:])
                nc.vector.tensor_mul(m2[:], m2[:], gna_sb[layer][:])
                nc.vector.tensor_sub(m2[:], m2[:], bconv_sb[layer][:])
                # c = x - m2 (per-partition scalar), full width
                nc.vector.tensor_scalar(
                    x_sb[:], x_sb[:], m2[:], None, op0=ALU.subtract
                )
                vcol = sp.tile([D, 1], F32, tag="vcol", name="vcol", bufs=1)
                nc.vector.memset(vcol[:], 0.0)
                pos = 0
                while pos < NL:
                    ln = min(cfg.MMCH, NL - pos)
                    sqs = sp.tile([128, cfg.MMCH], F32, tag="sqs", name="sqs")
                    nc.vector.tensor_mul(
                        sqs[:, :ln], x_sb[:, pos : pos + ln], x_sb[:, pos : pos + ln]
                    )
                    vnew = sp.tile([D, 1], F32, tag="vc", name="vc")
                    nc.vector.tensor_reduce(
                        vnew[:], sqs[:, :ln], axis=AXIS.X, op=ALU.add
                    )
                    nc.vector.tensor_add(vcol[:], vcol[:], vnew[:])
                    pos += ln
                gvar = allreduce(vcol[:], RS_IN2, RS_OUT2, "ar_var")
                vs = sp.tile([D, 1], F32, tag="vs", name="vs")
                nc.vector.tensor_scalar(
                    vs[:], gvar[:], 1.0 / N, EPS, op0=ALU.mult, op1=ALU.add
                )
                rc = sp.tile([D, 1], F32, tag="rc", name="rc")
                nc.vector.reciprocal(rc[:], vs[:])
                rstd = sp.tile([D, 1], F32, tag="rstd", name="rstd")
                nc.scalar.activation(rstd[:], rc[:], AF.Sqrt)
                f = sp.tile([D, 1], F32, tag="fcol", name="fcol")
                nc.vector.tensor_mul(f[:], rstd[:], gnw_sb[layer][:])
                # x = relu(c*f + gb)
                nc.vector.tensor_scalar(
                    x_sb[:], x_sb[:], f[:], gnb_sb[layer][:],
                    op0=ALU.mult, op1=ALU.add,
                )
                nc.scalar.activation(x_sb[:], x_sb[:], AF.Relu)

            def mlp_head():
                for k in range(NMM):
                    sl = slice(k * MMCH, (k + 1) * MMCH)
                    yp = ps_h.tile([128, MMCH], F32, tag="hp", name="m_yp")
                    nc.tensor.matmul(yp[:], lin0_sb[:], x_sb[:, sl], start=True, stop=True)
                    y = sp.tile([128, MMCH], F32, tag="m_y", name="m_y")
                    nc.vector.tensor_scalar(
                        y[:], yp[:], lin0b_sb[:], 0.0, op0=ALU.add, op1=ALU.max
                    )
                    op = ps_t.tile([1, MMCH], F32, tag="tp", name="m_op")
                    nc.tensor.matmul(op[:], lin1_sb[:], y[:], start=True, stop=True)
                    ob = sp.tile([1, MMCH], F32, tag="m_ob", name="m_ob")
                    nc.vector.tensor_scalar_add(ob[:], op[:], lin1b)
                    nc.sync.dma_start(OUT.ap()[:, sl], ob[:])

            for layer in range(2):
                prologue(layer)
                gather_and_aggregate(layer)
                graphnorm_relu(layer)
            mlp_head()

    nc.compile()
    return nc


def _make_const_inputs(cfg: Cfg, weights: dict):
    c = {}
    c["identb"] = np.eye(128, dtype=np.float32).astype(ml_dtypes.bfloat16)
    c["w0"] = np.asarray(weights["W0"], np.float32)
    c["w1"] = np.asarray(weights["W1"], np.float32)
    for l in range(2):
        c[f"gn{l}_a"] = np.asarray(weights[f"gn{l}_a"], np.float32).reshape(D, 1)
        c[f"gn{l}_w"] = np.asarray(weights[f"gn{l}_w"], np.float32).reshape(D, 1)
        c[f"gn{l}_b"] = np.asarray(weights[f"gn{l}_b"], np.float32).reshape(D, 1)
        c[f"b{l}"] = np.asarray(weights[f"b{l}"], np.float32).reshape(D, 1)
    c["lin0_w"] = np.asarray(weights["lin0_w"], np.float32)
    c["lin0_b"] = np.asarray(weights["lin0_b"], np.float32).reshape(D, 1)
    c["lin1_w"] = np.asarray(weights["lin1_w"], np.float32).reshape(D, 1)
    return c


def run(cfg: Cfg, x, edge_index, weights, trace=False):
    ins, meta = preprocess(cfg, edge_index)
    consts = _make_const_inputs(cfg, weights)
    x = np.asarray(x, np.float32)
    in_maps = []
    for c in range(NCORES):
        m = dict(ins[c])
        m.update(consts)
        xs = np.zeros((cfg.NLOC_PAD, D), np.float32)
        xs[: cfg.NLOC] = x[c * cfg.NLOC : (c + 1) * cfg.NLOC]
        m["xt"] = xs.T.copy()
        in_maps.append(m)
    nc = build(cfg, meta, float(np.asarray(weights["lin1_b"]).reshape(-1)[0]))
    res = run_bass_kernel_spmd(nc, in_maps, core_ids=list(range(NCORES)), trace=trace)
    out = np.concatenate(
        [res.results[c]["out"][0, : cfg.NLOC] for c in range(NCORES)], axis=0
    )
    return out.reshape(-1, 1), res


def kernel(**inputs) -> np.ndarray:
    cfg = Cfg(N=100000)
    weights = {
        k: np.asarray(v) for k, v in inputs.items() if k not in ("x", "edge_index")
    }
    out, _ = run(
        cfg, np.asarray(inputs["x"]), np.asarray(inputs["edge_index"]), weights
    )
    return out.astype(np.float32)
